# revision 29
# baseline (speedup 1.0000x reference)
"""Bahdanau-attention kernel for one TRN2 chip (8 NeuronCores, SPMD).

Math (per batch row b, sequence position s):
    att[b, s] = v . tanh(h_part[b] + enc[s, b, :] @ W_e)
    out[b, :] = softmax(att[b, :])        with h_part = hidden @ W_h + b_attn

Sharding: pure data-parallel over batch (B=32 -> 4 per core), no collectives.
Layout strategy: the big matmul contracts over H, which must live on SBUF
partitions; inputs are re-laid-out host-side so every device DMA is a single
contiguous block.  All matmuls run as float32r (full fp32 storage, reduced
PE mode, 1 cycle/row at free-dim >= 256 - 4x faster than plain fp32).
"""

import sys

sys.path.insert(0, "/opt/trn_rl_repo")

import numpy as np

from concourse import bacc, bass, mybir, tile
from concourse.bass_utils import run_bass_kernel_spmd

H = 512
DH = 4 * H            # 2048 (hidden feature dim)
B, S = 32, 2048
NCORES = 8
BC = B // NCORES      # 4 batch rows per core
KH = H // 128         # 4 contraction tiles over H
KD = DH // 128        # 16 contraction tiles over DH
NQ = H // 128         # 4 output quadrants of H
SBLK = 512            # sequence positions per block
NBLK = S // SBLK      # 4 blocks
F32 = mybir.dt.float32
F32R = mybir.dt.float32r
BF16 = mybir.dt.bfloat16
F8 = mybir.dt.float8e4
WE_SCALE = 64.0

_NC_CACHE = None


def _build():
    nc = bacc.Bacc(
        "TRN2", target_bir_lowering=False, debug=False, num_devices=NCORES
    )
    enc_d = nc.dram_tensor(
        "enc_t", [BC, NBLK, 128, KH, SBLK], F8, kind="ExternalInput"
    )
    hid_d = nc.dram_tensor("hid_t", [128, KD, BC], BF16, kind="ExternalInput")
    wh_d = nc.dram_tensor("w_h", [128, KD, H], BF16, kind="ExternalInput")
    we_d = nc.dram_tensor("w_e", [128, KH, H], F8, kind="ExternalInput")
    ba_d = nc.dram_tensor("b_attn", [128, NQ], F32, kind="ExternalInput")
    v_d = nc.dram_tensor("v", [128, NQ], BF16, kind="ExternalInput")
    id_d = nc.dram_tensor("ident", [BC, BC], F32, kind="ExternalInput")
    out_d = nc.dram_tensor("out", [BC, S], F32, kind="ExternalOutput")

    TANH = mybir.ActivationFunctionType.Tanh
    EXP = mybir.ActivationFunctionType.Exp
    COPY = mybir.ActivationFunctionType.Copy

    with tile.TileContext(nc) as tc:
        with (
            tc.tile_pool(name="const", bufs=1) as constp,
            tc.tile_pool(name="enc", bufs=6) as encp,
            tc.tile_pool(name="energy", bufs=6) as enp,
            tc.tile_pool(name="small", bufs=1) as smallp,
            tc.tile_pool(name="psum_e", bufs=7, space=bass.MemorySpace.PSUM) as pse,
            tc.tile_pool(name="psum_s", bufs=1, space=bass.MemorySpace.PSUM) as pss,
        ):
            we_sb = constp.tile([128, KH, H], F8)
            for k in range(KH):
                nc.scalar.dma_start(we_sb[:, k, :], we_d[:, k, :])
            wh_sb = constp.tile([128, KD, H], BF16)
            nc.scalar.dma_start(wh_sb[:, 0 : KD // 2, :], wh_d[:, 0 : KD // 2, :])
            hid_sb = constp.tile([128, KD, BC], BF16)
            nc.scalar.dma_start(hid_sb[:], hid_d[:])
            ba_sb = constp.tile([128, NQ], F32)
            nc.scalar.dma_start(ba_sb[:], ba_d[:])
            v_sb = constp.tile([128, NQ], BF16)
            nc.scalar.dma_start(v_sb[:], v_d[:])
            id_sb = constp.tile([BC, BC], F32)
            nc.scalar.dma_start(id_sb[:], id_d[:])

            hptb = constp.tile([128, NQ, BC], F32)
            ex = smallp.tile([128, S], F32)
            out_sb = smallp.tile([128, S], F32)
            esum = smallp.tile([128, NBLK], F32)
            ssum = smallp.tile([128, 1], F32)
            rs = smallp.tile([128, 1], F32)

            ps_small = pss.tile([128, SBLK], F32)

            # HAM pre-warm: ~3.5 us of dummy matmuls on zeroed scratch while
            # the first DMAs are still in flight, so real matmuls start at
            # full clock (K=8/8)
            warm = constp.tile([128, 512], BF16)
            nc.vector.memset(warm[:], 0.0)
            for _ in range(8):
                nc.tensor.matmul(
                    ps_small[:, :], warm[:, 0:128], warm[:], start=True, stop=True
                )

            blocks = [(b, s) for b in range(BC) for s in range(NBLK)]
            NBLOCKS = len(blocks)
            ets = {}
            epss = {}

            def load_block(i):
                b, sblk = blocks[i]
                et = encp.tile([128, KH, SBLK], F8)
                nc.sync.dma_start(et[:], enc_d[b, sblk])
                ets[i] = et

            def emit_emm(i, qs=None):
                b, sblk = blocks[i]
                if qs is None or qs[0] == 0:
                    epss[i] = []
                et = ets[i]
                eps4 = epss[i]
                for q in (qs if qs is not None else range(NQ)):
                    eps = pse.tile([128, SBLK], F32)
                    for j in range(KH // 2):
                        nc.tensor.matmul(
                            eps[:],
                            we_sb[:, 2 * j : 2 * j + 2, q * 128 : (q + 1) * 128],
                            et[:, 2 * j : 2 * j + 2, :],
                            start=(j == 0),
                            stop=(j == KH // 2 - 1),
                            perf_mode=mybir.MatmulPerfMode.DoubleRow,
                        )
                    eps4.append(eps)
                if qs is None or qs[-1] == NQ - 1:
                    ets.pop(i)

            def emit_tail(i):
                b, sblk = blocks[i]
                att_ps = ps_small[(i % 2) * 32 : (i % 2) * 32 + 1, :]
                for q in range(NQ):
                    eps = epss[i][q]
                    en = enp.tile([128, SBLK], BF16)
                    nc.scalar.activation(
                        en[:],
                        eps[:],
                        TANH,
                        bias=hptb[:, q, b : b + 1],
                        scale=1.0 / WE_SCALE,
                    )
                    nc.tensor.matmul(
                        att_ps,
                        v_sb[:, q : q + 1],
                        en[:],
                        start=(q == 0),
                        stop=(q == NQ - 1),
                    )
                del epss[i]

            def emit_exp(i):
                # exp of block i's logits (no max-sub: |logit| <= ||v||_1 ~ 18)
                # with a fused partial row-sum.  Deferred one block so it never
                # head-of-line-blocks the next block's tanh ops in the ACT FIFO.
                b, sblk = blocks[i]
                r0 = b * 32
                att_ps = ps_small[(i % 2) * 32 : (i % 2) * 32 + 1, :]
                nc.scalar.activation(
                    ex[r0 : r0 + 1, sblk * SBLK : (sblk + 1) * SBLK],
                    att_ps,
                    EXP,
                )
                nc.vector.reduce_sum(
                    esum[r0 : r0 + 1, sblk : sblk + 1],
                    ex[r0 : r0 + 1, sblk * SBLK : (sblk + 1) * SBLK],
                    axis=mybir.AxisListType.X,
                )
                if sblk == NBLK - 1:
                    # normalize row b as soon as its blocks are done
                    nc.vector.reduce_sum(
                        ssum[r0 : r0 + 1, :],
                        esum[r0 : r0 + 1, :],
                        axis=mybir.AxisListType.X,
                    )
                    nc.vector.reciprocal(rs[r0 : r0 + 1, :], ssum[r0 : r0 + 1, :])
                    if i == NBLOCKS - 1:
                        # last row: split across engines so the exposed tail
                        # is half as long
                        hs = S // 2
                        nc.vector.tensor_scalar_mul(
                            out_sb[r0 : r0 + 1, 0:hs],
                            ex[r0 : r0 + 1, 0:hs],
                            rs[r0 : r0 + 1, :],
                        )
                        nc.scalar.activation(
                            out_sb[r0 : r0 + 1, hs:S],
                            ex[r0 : r0 + 1, hs:S],
                            COPY,
                            scale=rs[r0 : r0 + 1, :],
                        )
                        nc.sync.dma_start(
                            out_d[b : b + 1, 0:hs], out_sb[r0 : r0 + 1, 0:hs]
                        )
                        nc.scalar.dma_start(
                            out_d[b : b + 1, hs:S], out_sb[r0 : r0 + 1, hs:S]
                        )
                    else:
                        nc.vector.tensor_scalar_mul(
                            out_sb[r0 : r0 + 1, :],
                            ex[r0 : r0 + 1, :],
                            rs[r0 : r0 + 1, :],
                        )
                        nc.sync.dma_start(
                            out_d[b : b + 1, :], out_sb[r0 : r0 + 1, :]
                        )

            # prologue: sync queue carries only enc tiles (fp8, 256 KB each);
            # h_part matmuls interleave with block 0's e-matmuls so the tanh
            # bias is ready as early as possible
            load_block(0)
            nc.sync.dma_start(wh_sb[:, KD // 2 :, :], wh_d[:, KD // 2 :, :])
            load_block(1)
            hp_ps = ps_small[0:BC, 0:H]

            def emit_hp(ks):
                for k in ks:
                    nc.tensor.matmul(
                        hp_ps,
                        hid_sb[:, k, :],
                        wh_sb[:, k, :],
                        start=(k == 0),
                        stop=(k == KD - 1),
                    )

            emit_emm(0, qs=[0, 1])
            emit_hp(range(0, KD // 2))
            emit_emm(0, qs=[2, 3])
            emit_hp(range(KD // 2, KD))
            hp_sb = smallp.tile([BC, H], F32)
            nc.vector.tensor_copy(hp_sb[:], hp_ps)

            # transpose to [128, q, b] via PE, fold in b_attn -> tanh bias
            for q in range(NQ):
                hpt_ps = ps_small[:, q * BC : (q + 1) * BC]
                nc.tensor.transpose(
                    hpt_ps, hp_sb[:, q * 128 : (q + 1) * 128], id_sb[:]
                )
                nc.vector.tensor_scalar_add(
                    hptb[:, q, :], hpt_ps, ba_sb[:, q : q + 1]
                )

            # steady state, one-block skew: e-matmuls of block i+1 sit ahead of
            # block i's tanh-dependent v-dots in the PE stream
            for i in range(NBLOCKS):
                if i + 2 < NBLOCKS:
                    load_block(i + 2)
                if i + 1 < NBLOCKS:
                    emit_emm(i + 1)
                emit_tail(i)
                if i == NBLOCKS - 1:
                    emit_exp(i - 1)
                    emit_exp(i)
                elif i > 0:
                    emit_exp(i - 1)

    nc.compile()
    return nc


def _get_nc():
    global _NC_CACHE
    if _NC_CACHE is None:
        _NC_CACHE = _build()
    return _NC_CACHE


def _prep_inputs(hidden, encoder_outputs, W_attn, b_attn, v):
    f = np.float32
    W_h = np.asarray(W_attn[:DH], dtype=f)
    W_e = np.asarray(W_attn[DH:], dtype=f)
    import ml_dtypes
    bf = ml_dtypes.bfloat16
    f8 = ml_dtypes.float8_e4m3
    wh_prep = np.ascontiguousarray(W_h.reshape(KD, 128, H).transpose(1, 0, 2)).astype(bf)
    we_prep = np.clip(
        np.ascontiguousarray(W_e.reshape(KH, 128, H).transpose(1, 0, 2)) * 64.0,
        -240.0, 240.0,
    ).astype(f8)
    ba_prep = np.ascontiguousarray(np.asarray(b_attn, dtype=f).reshape(NQ, 128).T)
    v_prep = np.ascontiguousarray(np.asarray(v, dtype=f).reshape(NQ, 128).T).astype(bf)
    ident = np.eye(BC, dtype=f)
    hidden = np.asarray(hidden, dtype=f)
    encoder_outputs = np.asarray(encoder_outputs, dtype=f)

    in_maps = []
    for c in range(NCORES):
        b0 = c * BC
        hc = hidden[b0 : b0 + BC]                       # [BC, DH]
        hid_prep = np.ascontiguousarray(
            hc.T.reshape(KD, 128, BC).transpose(1, 0, 2)
        ).astype(bf)
        ec = encoder_outputs[:, b0 : b0 + BC, :]        # [S, BC, H]
        # enc_prep[b, sblk, p, k, si] = ec[sblk*SBLK+si, b, k*128+p]
        enc_prep = np.clip(
            np.ascontiguousarray(
                ec.transpose(1, 0, 2)
                .reshape(BC, NBLK, SBLK, KH, 128)
                .transpose(0, 1, 4, 3, 2)
            ),
            -240.0, 240.0,
        ).astype(ml_dtypes.float8_e4m3)
        in_maps.append(
            {
                "enc_t": enc_prep,
                "hid_t": hid_prep,
                "w_h": wh_prep,
                "w_e": we_prep,
                "b_attn": ba_prep,
                "v": v_prep,
                "ident": ident,
            }
        )
    return in_maps


def _run(inputs, trace=False, **kw):
    nc = _get_nc()
    in_maps = _prep_inputs(
        inputs["hidden"],
        inputs["encoder_outputs"],
        inputs["W_attn"],
        inputs["b_attn"],
        inputs["v"],
    )
    res = run_bass_kernel_spmd(
        nc, in_maps, core_ids=list(range(NCORES)), trace=trace, **kw
    )
    out = np.concatenate([r["out"] for r in res.results], axis=0).astype(np.float32)
    return out, res


def kernel(**inputs):
    out, _ = _run(inputs, trace=False)
    return out


# revision 30
# speedup vs baseline: 1.0648x; 1.0648x over previous
"""Bahdanau-attention kernel for one TRN2 chip (8 NeuronCores, SPMD).

Math (per batch row b, sequence position s):
    att[b, s] = v . tanh(h_part[b] + enc[s, b, :] @ W_e)
    out[b, :] = softmax(att[b, :])        with h_part = hidden @ W_h + b_attn

Sharding: pure data-parallel over batch (B=32 -> 4 per core), no collectives.
Layout strategy: the big matmul contracts over H, which must live on SBUF
partitions; inputs are re-laid-out host-side so every device DMA is a single
contiguous block.  All matmuls run as float32r (full fp32 storage, reduced
PE mode, 1 cycle/row at free-dim >= 256 - 4x faster than plain fp32).
"""

import sys

sys.path.insert(0, "/opt/trn_rl_repo")

import numpy as np

from concourse import bacc, bass, mybir, tile
from concourse.bass_utils import run_bass_kernel_spmd

H = 512
DH = 4 * H            # 2048 (hidden feature dim)
B, S = 32, 2048
NCORES = 8
BC = B // NCORES      # 4 batch rows per core
KH = H // 128         # 4 contraction tiles over H
KD = DH // 128        # 16 contraction tiles over DH
NQ = H // 128         # 4 output quadrants of H
SBLK = 512            # sequence positions per block
NBLK = S // SBLK      # 4 blocks
F32 = mybir.dt.float32
F32R = mybir.dt.float32r
BF16 = mybir.dt.bfloat16
F8 = mybir.dt.float8e4
WE_SCALE = 64.0

_NC_CACHE = None


def _build():
    nc = bacc.Bacc(
        "TRN2", target_bir_lowering=False, debug=False, num_devices=NCORES
    )
    enc_d = nc.dram_tensor(
        "enc_t", [BC, NBLK, 128, KH, SBLK], F8, kind="ExternalInput"
    )
    hid_d = nc.dram_tensor("hid_t", [128, KD, BC], BF16, kind="ExternalInput")
    wh_d = nc.dram_tensor("w_h", [128, KD, H], BF16, kind="ExternalInput")
    we_d = nc.dram_tensor("w_e", [128, KH, H], F8, kind="ExternalInput")
    ba_d = nc.dram_tensor("b_attn", [128, NQ], F32, kind="ExternalInput")
    v_d = nc.dram_tensor("v", [128, NQ], BF16, kind="ExternalInput")
    id_d = nc.dram_tensor("ident", [BC, BC], F32, kind="ExternalInput")
    out_d = nc.dram_tensor("out", [BC, S], F32, kind="ExternalOutput")

    TANH = mybir.ActivationFunctionType.Tanh
    EXP = mybir.ActivationFunctionType.Exp
    COPY = mybir.ActivationFunctionType.Copy

    with tile.TileContext(nc) as tc:
        with (
            tc.tile_pool(name="const", bufs=1) as constp,
            tc.tile_pool(name="enc", bufs=6) as encp,
            tc.tile_pool(name="energy", bufs=9) as enp,
            tc.tile_pool(name="small", bufs=1) as smallp,
            tc.tile_pool(name="psum_e", bufs=7, space=bass.MemorySpace.PSUM) as pse,
            tc.tile_pool(name="psum_s", bufs=1, space=bass.MemorySpace.PSUM) as pss,
        ):
            we_sb = constp.tile([128, KH, H], F8)
            for k in range(KH):
                nc.scalar.dma_start(we_sb[:, k, :], we_d[:, k, :])
            wh_sb = constp.tile([128, KD, H], BF16)
            nc.scalar.dma_start(wh_sb[:, 0 : KD // 2, :], wh_d[:, 0 : KD // 2, :])
            hid_sb = constp.tile([128, KD, BC], BF16)
            nc.scalar.dma_start(hid_sb[:], hid_d[:])
            ba_sb = constp.tile([128, NQ], F32)
            nc.scalar.dma_start(ba_sb[:], ba_d[:])
            v_sb = constp.tile([128, NQ], BF16)
            nc.scalar.dma_start(v_sb[:], v_d[:])
            id_sb = constp.tile([BC, BC], F32)
            nc.scalar.dma_start(id_sb[:], id_d[:])

            hptb = constp.tile([128, NQ, BC], F32)
            ex = smallp.tile([128, S], F32)
            out_sb = smallp.tile([128, S], F32)
            esum = smallp.tile([128, NBLK], F32)
            ssum = smallp.tile([128, 1], F32)
            rs = smallp.tile([128, 1], F32)

            ps_small = pss.tile([128, SBLK], F32)

            # HAM pre-warm: ~3.5 us of dummy matmuls on zeroed scratch while
            # the first DMAs are still in flight, so real matmuls start at
            # full clock (K=8/8)
            warm = constp.tile([128, 512], BF16)
            nc.vector.memset(warm[:], 0.0)
            for _ in range(8):
                nc.tensor.matmul(
                    ps_small[:, :], warm[:, 0:128], warm[:], start=True, stop=True
                )

            blocks = [(b, s) for b in range(BC) for s in range(NBLK)]
            NBLOCKS = len(blocks)
            ets = {}
            epss = {}

            def load_block(i):
                b, sblk = blocks[i]
                et = encp.tile([128, KH, SBLK], F8)
                nc.sync.dma_start(et[:], enc_d[b, sblk])
                ets[i] = et

            def emit_emm(i, qs=None):
                b, sblk = blocks[i]
                if qs is None or qs[0] == 0:
                    epss[i] = []
                et = ets[i]
                eps4 = epss[i]
                for q in (qs if qs is not None else range(NQ)):
                    eps = pse.tile([128, SBLK], F32)
                    for j in range(KH // 2):
                        nc.tensor.matmul(
                            eps[:],
                            we_sb[:, 2 * j : 2 * j + 2, q * 128 : (q + 1) * 128],
                            et[:, 2 * j : 2 * j + 2, :],
                            start=(j == 0),
                            stop=(j == KH // 2 - 1),
                            perf_mode=mybir.MatmulPerfMode.DoubleRow,
                        )
                    eps4.append(eps)
                if qs is None or qs[-1] == NQ - 1:
                    ets.pop(i)

            ens = {}

            def emit_tanh(i):
                b, sblk = blocks[i]
                en4 = []
                for q in range(NQ):
                    eps = epss[i][q]
                    en = enp.tile([128, SBLK], BF16)
                    nc.scalar.activation(
                        en[:],
                        eps[:],
                        TANH,
                        bias=hptb[:, q, b : b + 1],
                        scale=1.0 / WE_SCALE,
                    )
                    en4.append(en)
                ens[i] = en4
                del epss[i]

            def emit_v(i):
                att_ps = ps_small[(i % 3) * 32 : (i % 3) * 32 + 1, :]
                for q in range(NQ):
                    nc.tensor.matmul(
                        att_ps,
                        v_sb[:, q : q + 1],
                        ens[i][q][:],
                        start=(q == 0),
                        stop=(q == NQ - 1),
                    )
                del ens[i]

            def emit_exp(i):
                # exp of block i's logits (no max-sub: |logit| <= ||v||_1 ~ 18)
                # with a fused partial row-sum.  Deferred one block so it never
                # head-of-line-blocks the next block's tanh ops in the ACT FIFO.
                b, sblk = blocks[i]
                r0 = b * 32
                att_ps = ps_small[(i % 3) * 32 : (i % 3) * 32 + 1, :]
                nc.scalar.activation(
                    ex[r0 : r0 + 1, sblk * SBLK : (sblk + 1) * SBLK],
                    att_ps,
                    EXP,
                )
                nc.vector.reduce_sum(
                    esum[r0 : r0 + 1, sblk : sblk + 1],
                    ex[r0 : r0 + 1, sblk * SBLK : (sblk + 1) * SBLK],
                    axis=mybir.AxisListType.X,
                )
                if sblk == NBLK - 1:
                    # normalize row b as soon as its blocks are done
                    nc.vector.reduce_sum(
                        ssum[r0 : r0 + 1, :],
                        esum[r0 : r0 + 1, :],
                        axis=mybir.AxisListType.X,
                    )
                    nc.vector.reciprocal(rs[r0 : r0 + 1, :], ssum[r0 : r0 + 1, :])
                    if i == NBLOCKS - 1:
                        # last row: split across engines so the exposed tail
                        # is half as long
                        hs = S // 2
                        nc.vector.tensor_scalar_mul(
                            out_sb[r0 : r0 + 1, 0:hs],
                            ex[r0 : r0 + 1, 0:hs],
                            rs[r0 : r0 + 1, :],
                        )
                        nc.scalar.activation(
                            out_sb[r0 : r0 + 1, hs:S],
                            ex[r0 : r0 + 1, hs:S],
                            COPY,
                            scale=rs[r0 : r0 + 1, :],
                        )
                        nc.sync.dma_start(
                            out_d[b : b + 1, 0:hs], out_sb[r0 : r0 + 1, 0:hs]
                        )
                        nc.scalar.dma_start(
                            out_d[b : b + 1, hs:S], out_sb[r0 : r0 + 1, hs:S]
                        )
                    else:
                        nc.vector.tensor_scalar_mul(
                            out_sb[r0 : r0 + 1, :],
                            ex[r0 : r0 + 1, :],
                            rs[r0 : r0 + 1, :],
                        )
                        nc.sync.dma_start(
                            out_d[b : b + 1, :], out_sb[r0 : r0 + 1, :]
                        )

            # prologue: sync queue carries only enc tiles (fp8, 256 KB each);
            # h_part matmuls interleave with block 0's e-matmuls so the tanh
            # bias is ready as early as possible
            load_block(0)
            nc.sync.dma_start(wh_sb[:, KD // 2 :, :], wh_d[:, KD // 2 :, :])
            load_block(1)
            hp_ps = ps_small[0:BC, 0:H]

            def emit_hp(ks):
                for k in ks:
                    nc.tensor.matmul(
                        hp_ps,
                        hid_sb[:, k, :],
                        wh_sb[:, k, :],
                        start=(k == 0),
                        stop=(k == KD - 1),
                    )

            emit_emm(0, qs=[0, 1])
            emit_hp(range(0, KD // 2))
            emit_emm(0, qs=[2, 3])
            emit_hp(range(KD // 2, KD))
            hp_sb = smallp.tile([BC, H], F32)
            nc.vector.tensor_copy(hp_sb[:], hp_ps)

            # transpose to [128, q, b] via PE, fold in b_attn -> tanh bias
            for q in range(NQ):
                hpt_ps = ps_small[:, q * BC : (q + 1) * BC]
                nc.tensor.transpose(
                    hpt_ps, hp_sb[:, q * 128 : (q + 1) * 128], id_sb[:]
                )
                nc.vector.tensor_scalar_add(
                    hptb[:, q, :], hpt_ps, ba_sb[:, q : q + 1]
                )

            # steady state, one-block skew: e-matmuls of block i+1 sit ahead of
            # block i's tanh-dependent v-dots in the PE stream
            for i in range(NBLOCKS):
                if i + 2 < NBLOCKS:
                    load_block(i + 2)
                if i + 1 < NBLOCKS:
                    emit_emm(i + 1)
                emit_tanh(i)
                if i >= 1:
                    emit_v(i - 1)
                if i >= 2:
                    emit_exp(i - 2)
            emit_v(NBLOCKS - 1)
            emit_exp(NBLOCKS - 2)
            emit_exp(NBLOCKS - 1)

    nc.compile()
    return nc


def _get_nc():
    global _NC_CACHE
    if _NC_CACHE is None:
        _NC_CACHE = _build()
    return _NC_CACHE


def _prep_inputs(hidden, encoder_outputs, W_attn, b_attn, v):
    f = np.float32
    W_h = np.asarray(W_attn[:DH], dtype=f)
    W_e = np.asarray(W_attn[DH:], dtype=f)
    import ml_dtypes
    bf = ml_dtypes.bfloat16
    f8 = ml_dtypes.float8_e4m3
    wh_prep = np.ascontiguousarray(W_h.reshape(KD, 128, H).transpose(1, 0, 2)).astype(bf)
    we_prep = np.clip(
        np.ascontiguousarray(W_e.reshape(KH, 128, H).transpose(1, 0, 2)) * 64.0,
        -240.0, 240.0,
    ).astype(f8)
    ba_prep = np.ascontiguousarray(np.asarray(b_attn, dtype=f).reshape(NQ, 128).T)
    v_prep = np.ascontiguousarray(np.asarray(v, dtype=f).reshape(NQ, 128).T).astype(bf)
    ident = np.eye(BC, dtype=f)
    hidden = np.asarray(hidden, dtype=f)
    encoder_outputs = np.asarray(encoder_outputs, dtype=f)

    in_maps = []
    for c in range(NCORES):
        b0 = c * BC
        hc = hidden[b0 : b0 + BC]                       # [BC, DH]
        hid_prep = np.ascontiguousarray(
            hc.T.reshape(KD, 128, BC).transpose(1, 0, 2)
        ).astype(bf)
        ec = encoder_outputs[:, b0 : b0 + BC, :]        # [S, BC, H]
        # enc_prep[b, sblk, p, k, si] = ec[sblk*SBLK+si, b, k*128+p]
        enc_prep = np.clip(
            np.ascontiguousarray(
                ec.transpose(1, 0, 2)
                .reshape(BC, NBLK, SBLK, KH, 128)
                .transpose(0, 1, 4, 3, 2)
            ),
            -240.0, 240.0,
        ).astype(ml_dtypes.float8_e4m3)
        in_maps.append(
            {
                "enc_t": enc_prep,
                "hid_t": hid_prep,
                "w_h": wh_prep,
                "w_e": we_prep,
                "b_attn": ba_prep,
                "v": v_prep,
                "ident": ident,
            }
        )
    return in_maps


def _run(inputs, trace=False, **kw):
    nc = _get_nc()
    in_maps = _prep_inputs(
        inputs["hidden"],
        inputs["encoder_outputs"],
        inputs["W_attn"],
        inputs["b_attn"],
        inputs["v"],
    )
    res = run_bass_kernel_spmd(
        nc, in_maps, core_ids=list(range(NCORES)), trace=trace, **kw
    )
    out = np.concatenate([r["out"] for r in res.results], axis=0).astype(np.float32)
    return out, res


def kernel(**inputs):
    out, _ = _run(inputs, trace=False)
    return out


# revision 33
# speedup vs baseline: 1.0839x; 1.0180x over previous
"""Bahdanau-attention kernel for one TRN2 chip (8 NeuronCores, SPMD).

Math (per batch row b, sequence position s):
    att[b, s] = v . tanh(h_part[b] + enc[s, b, :] @ W_e)
    out[b, :] = softmax(att[b, :])        with h_part = hidden @ W_h + b_attn

Sharding: pure data-parallel over batch (B=32 -> 4 per core), no collectives.
Layout strategy: the big matmul contracts over H, which must live on SBUF
partitions; inputs are re-laid-out host-side so every device DMA is a single
contiguous block.  All matmuls run as float32r (full fp32 storage, reduced
PE mode, 1 cycle/row at free-dim >= 256 - 4x faster than plain fp32).
"""

import sys

sys.path.insert(0, "/opt/trn_rl_repo")

import numpy as np

from concourse import bacc, bass, mybir, tile
from concourse.bass_utils import run_bass_kernel_spmd

H = 512
DH = 4 * H            # 2048 (hidden feature dim)
B, S = 32, 2048
NCORES = 8
BC = B // NCORES      # 4 batch rows per core
KH = H // 128         # 4 contraction tiles over H
KD = DH // 128        # 16 contraction tiles over DH
NQ = H // 128         # 4 output quadrants of H
SBLK = 512            # sequence positions per block
NBLK = S // SBLK      # 4 blocks
F32 = mybir.dt.float32
F32R = mybir.dt.float32r
BF16 = mybir.dt.bfloat16
F8 = mybir.dt.float8e4
WE_SCALE = 64.0

_NC_CACHE = None


def _build():
    nc = bacc.Bacc(
        "TRN2", target_bir_lowering=False, debug=False, num_devices=NCORES
    )
    enc_d = nc.dram_tensor(
        "enc_t", [BC, NBLK, 128, KH, SBLK], F8, kind="ExternalInput"
    )
    hid_d = nc.dram_tensor("hid_t", [128, KD, BC], BF16, kind="ExternalInput")
    wh_d = nc.dram_tensor("w_h", [128, KD, H], BF16, kind="ExternalInput")
    we_d = nc.dram_tensor("w_e", [128, KH, H], F8, kind="ExternalInput")
    ba_d = nc.dram_tensor("b_attn", [128, NQ], F32, kind="ExternalInput")
    v_d = nc.dram_tensor("v", [128, NQ], BF16, kind="ExternalInput")
    id_d = nc.dram_tensor("ident", [BC, BC], F32, kind="ExternalInput")
    out_d = nc.dram_tensor("out", [BC, S], F32, kind="ExternalOutput")

    TANH = mybir.ActivationFunctionType.Tanh
    EXP = mybir.ActivationFunctionType.Exp
    COPY = mybir.ActivationFunctionType.Copy

    with tile.TileContext(nc) as tc:
        with (
            tc.tile_pool(name="const", bufs=1) as constp,
            tc.tile_pool(name="enc", bufs=6) as encp,
            tc.tile_pool(name="energy", bufs=9) as enp,
            tc.tile_pool(name="small", bufs=1) as smallp,
            tc.tile_pool(name="psum_e", bufs=7, space=bass.MemorySpace.PSUM) as pse,
            tc.tile_pool(name="psum_s", bufs=1, space=bass.MemorySpace.PSUM) as pss,
        ):
            we_sb = constp.tile([128, KH, H], F8)
            for k in range(KH):
                nc.scalar.dma_start(we_sb[:, k, :], we_d[:, k, :])
            wh_sb = constp.tile([128, KD, H], BF16)
            nc.scalar.dma_start(wh_sb[:, 0 : KD // 2, :], wh_d[:, 0 : KD // 2, :])
            hid_sb = constp.tile([128, KD, BC], BF16)
            nc.scalar.dma_start(hid_sb[:], hid_d[:])
            ba_sb = constp.tile([128, NQ], F32)
            nc.scalar.dma_start(ba_sb[:], ba_d[:])
            v_sb = constp.tile([128, NQ], BF16)
            nc.scalar.dma_start(v_sb[:], v_d[:])
            id_sb = constp.tile([BC, BC], F32)
            nc.scalar.dma_start(id_sb[:], id_d[:])

            hptb = constp.tile([128, NQ, BC], F32)
            ex = smallp.tile([128, S], F32)
            out_sb = smallp.tile([128, S], F32)
            esum = smallp.tile([128, NBLK], F32)
            ssum = smallp.tile([128, 1], F32)
            rs = smallp.tile([128, 1], F32)

            ps_small = pss.tile([128, SBLK], F32)

            # HAM pre-warm: ~3.5 us of dummy matmuls on zeroed scratch while
            # the first DMAs are still in flight, so real matmuls start at
            # full clock (K=8/8)
            warm = constp.tile([128, 512], BF16)
            nc.vector.memset(warm[:], 0.0)
            for _ in range(8):
                nc.tensor.matmul(
                    ps_small[:, :], warm[:, 0:128], warm[:], start=True, stop=True
                )

            blocks = [(b, s) for b in range(BC) for s in range(NBLK)]
            NBLOCKS = len(blocks)
            ets = {}
            epss = {}

            def load_block(i):
                b, sblk = blocks[i]
                et = encp.tile([128, KH, SBLK], F8)
                nc.sync.dma_start(et[:], enc_d[b, sblk])
                ets[i] = et

            def emit_emm(i, qs=None):
                b, sblk = blocks[i]
                if qs is None or qs[0] == 0:
                    epss[i] = []
                et = ets[i]
                eps4 = epss[i]
                qlist = list(qs) if qs is not None else list(range(NQ))
                tiles = {}
                for q in qlist:
                    tiles[q] = pse.tile([128, SBLK], F32, name="eps", tag="eps")
                for qpair in [qlist[i : i + 2] for i in range(0, len(qlist), 2)]:
                    for j in range(KH // 2):
                        for q in qpair:
                            nc.tensor.matmul(
                                tiles[q][:],
                                we_sb[:, 2 * j : 2 * j + 2, q * 128 : (q + 1) * 128],
                                et[:, 2 * j : 2 * j + 2, :],
                                start=(j == 0),
                                stop=(j == KH // 2 - 1),
                                perf_mode=mybir.MatmulPerfMode.DoubleRow,
                            )
                for q in qlist:
                    eps4.append(tiles[q])
                if qs is None or qs[-1] == NQ - 1:
                    ets.pop(i)

            ens = {}

            def emit_tanh(i):
                b, sblk = blocks[i]
                en4 = []
                for q in range(NQ):
                    eps = epss[i][q]
                    en = enp.tile([128, SBLK], BF16)
                    nc.scalar.activation(
                        en[:],
                        eps[:],
                        TANH,
                        bias=hptb[:, q, b : b + 1],
                        scale=1.0 / WE_SCALE,
                    )
                    en4.append(en)
                ens[i] = en4
                del epss[i]

            def emit_v(i):
                att_ps = ps_small[(i % 3) * 32 : (i % 3) * 32 + 1, :]
                for q in range(NQ):
                    nc.tensor.matmul(
                        att_ps,
                        v_sb[:, q : q + 1],
                        ens[i][q][:],
                        start=(q == 0),
                        stop=(q == NQ - 1),
                    )
                del ens[i]

            def emit_exp(i):
                # exp of block i's logits (no max-sub: |logit| <= ||v||_1 ~ 18)
                # with a fused partial row-sum.  Deferred one block so it never
                # head-of-line-blocks the next block's tanh ops in the ACT FIFO.
                b, sblk = blocks[i]
                r0 = b * 32
                att_ps = ps_small[(i % 3) * 32 : (i % 3) * 32 + 1, :]
                if i >= NBLOCKS - 2:
                    # tail-critical: fused accumulator (290 ns) beats a
                    # separate 680 ns single-partition DVE reduce
                    nc.scalar.activation(
                        ex[r0 : r0 + 1, sblk * SBLK : (sblk + 1) * SBLK],
                        att_ps,
                        EXP,
                        accum_out=esum[r0 : r0 + 1, sblk : sblk + 1],
                    )
                else:
                    nc.scalar.activation(
                        ex[r0 : r0 + 1, sblk * SBLK : (sblk + 1) * SBLK],
                        att_ps,
                        EXP,
                    )
                    nc.vector.reduce_sum(
                        esum[r0 : r0 + 1, sblk : sblk + 1],
                        ex[r0 : r0 + 1, sblk * SBLK : (sblk + 1) * SBLK],
                        axis=mybir.AxisListType.X,
                    )
                if sblk == NBLK - 1:
                    # normalize row b as soon as its blocks are done
                    nc.vector.reduce_sum(
                        ssum[r0 : r0 + 1, :],
                        esum[r0 : r0 + 1, :],
                        axis=mybir.AxisListType.X,
                    )
                    nc.vector.reciprocal(rs[r0 : r0 + 1, :], ssum[r0 : r0 + 1, :])
                    if i == NBLOCKS - 1:
                        # last row: split across engines so the exposed tail
                        # is half as long
                        hs = S // 2
                        nc.vector.tensor_scalar_mul(
                            out_sb[r0 : r0 + 1, 0:hs],
                            ex[r0 : r0 + 1, 0:hs],
                            rs[r0 : r0 + 1, :],
                        )
                        nc.scalar.activation(
                            out_sb[r0 : r0 + 1, hs:S],
                            ex[r0 : r0 + 1, hs:S],
                            COPY,
                            scale=rs[r0 : r0 + 1, :],
                        )
                        nc.sync.dma_start(
                            out_d[b : b + 1, 0:hs], out_sb[r0 : r0 + 1, 0:hs]
                        )
                        nc.scalar.dma_start(
                            out_d[b : b + 1, hs:S], out_sb[r0 : r0 + 1, hs:S]
                        )
                    else:
                        nc.vector.tensor_scalar_mul(
                            out_sb[r0 : r0 + 1, :],
                            ex[r0 : r0 + 1, :],
                            rs[r0 : r0 + 1, :],
                        )
                        nc.sync.dma_start(
                            out_d[b : b + 1, :], out_sb[r0 : r0 + 1, :]
                        )

            # prologue: sync queue carries only enc tiles (fp8, 256 KB each);
            # h_part matmuls interleave with block 0's e-matmuls so the tanh
            # bias is ready as early as possible
            load_block(0)
            nc.sync.dma_start(wh_sb[:, KD // 2 :, :], wh_d[:, KD // 2 :, :])
            load_block(1)
            hp_ps = ps_small[0:BC, 0:H]

            def emit_hp(ks):
                for k in ks:
                    nc.tensor.matmul(
                        hp_ps,
                        hid_sb[:, k, :],
                        wh_sb[:, k, :],
                        start=(k == 0),
                        stop=(k == KD - 1),
                    )

            emit_emm(0, qs=[0, 1])
            emit_hp(range(0, KD // 2))
            emit_emm(0, qs=[2, 3])
            emit_hp(range(KD // 2, KD))
            hp_sb = smallp.tile([BC, H], F32)
            nc.vector.tensor_copy(hp_sb[:], hp_ps)

            # transpose to [128, q, b] via PE, fold in b_attn -> tanh bias
            for q in range(NQ):
                hpt_ps = ps_small[:, q * BC : (q + 1) * BC]
                nc.tensor.transpose(
                    hpt_ps, hp_sb[:, q * 128 : (q + 1) * 128], id_sb[:]
                )
                nc.vector.tensor_scalar_add(
                    hptb[:, q, :], hpt_ps, ba_sb[:, q : q + 1]
                )

            # steady state, one-block skew: e-matmuls of block i+1 sit ahead of
            # block i's tanh-dependent v-dots in the PE stream
            for i in range(NBLOCKS):
                if i + 2 < NBLOCKS:
                    load_block(i + 2)
                if i + 1 < NBLOCKS:
                    emit_emm(i + 1)
                emit_tanh(i)
                if i >= 1:
                    emit_v(i - 1)
                if i >= 2:
                    emit_exp(i - 2)
            emit_exp(NBLOCKS - 2)
            emit_v(NBLOCKS - 1)
            emit_exp(NBLOCKS - 1)

    nc.compile()
    return nc


def _get_nc():
    global _NC_CACHE
    if _NC_CACHE is None:
        _NC_CACHE = _build()
    return _NC_CACHE


def _prep_inputs(hidden, encoder_outputs, W_attn, b_attn, v):
    f = np.float32
    W_h = np.asarray(W_attn[:DH], dtype=f)
    W_e = np.asarray(W_attn[DH:], dtype=f)
    import ml_dtypes
    bf = ml_dtypes.bfloat16
    f8 = ml_dtypes.float8_e4m3
    wh_prep = np.ascontiguousarray(W_h.reshape(KD, 128, H).transpose(1, 0, 2)).astype(bf)
    we_prep = np.clip(
        np.ascontiguousarray(W_e.reshape(KH, 128, H).transpose(1, 0, 2)) * 64.0,
        -240.0, 240.0,
    ).astype(f8)
    ba_prep = np.ascontiguousarray(np.asarray(b_attn, dtype=f).reshape(NQ, 128).T)
    v_prep = np.ascontiguousarray(np.asarray(v, dtype=f).reshape(NQ, 128).T).astype(bf)
    ident = np.eye(BC, dtype=f)
    hidden = np.asarray(hidden, dtype=f)
    encoder_outputs = np.asarray(encoder_outputs, dtype=f)

    in_maps = []
    for c in range(NCORES):
        b0 = c * BC
        hc = hidden[b0 : b0 + BC]                       # [BC, DH]
        hid_prep = np.ascontiguousarray(
            hc.T.reshape(KD, 128, BC).transpose(1, 0, 2)
        ).astype(bf)
        ec = encoder_outputs[:, b0 : b0 + BC, :]        # [S, BC, H]
        # enc_prep[b, sblk, p, k, si] = ec[sblk*SBLK+si, b, k*128+p]
        enc_prep = np.clip(
            np.ascontiguousarray(
                ec.transpose(1, 0, 2)
                .reshape(BC, NBLK, SBLK, KH, 128)
                .transpose(0, 1, 4, 3, 2)
            ),
            -240.0, 240.0,
        ).astype(ml_dtypes.float8_e4m3)
        in_maps.append(
            {
                "enc_t": enc_prep,
                "hid_t": hid_prep,
                "w_h": wh_prep,
                "w_e": we_prep,
                "b_attn": ba_prep,
                "v": v_prep,
                "ident": ident,
            }
        )
    return in_maps


def _run(inputs, trace=False, **kw):
    nc = _get_nc()
    in_maps = _prep_inputs(
        inputs["hidden"],
        inputs["encoder_outputs"],
        inputs["W_attn"],
        inputs["b_attn"],
        inputs["v"],
    )
    res = run_bass_kernel_spmd(
        nc, in_maps, core_ids=list(range(NCORES)), trace=trace, **kw
    )
    out = np.concatenate([r["out"] for r in res.results], axis=0).astype(np.float32)
    return out, res


def kernel(**inputs):
    out, _ = _run(inputs, trace=False)
    return out


# revision 34
# speedup vs baseline: 1.1201x; 1.0333x over previous
"""Bahdanau-attention kernel for one TRN2 chip (8 NeuronCores, SPMD).

Math (per batch row b, sequence position s):
    att[b, s] = v . tanh(h_part[b] + enc[s, b, :] @ W_e)
    out[b, :] = softmax(att[b, :])        with h_part = hidden @ W_h + b_attn

Sharding: pure data-parallel over batch (B=32 -> 4 per core), no collectives.
Layout strategy: the big matmul contracts over H, which must live on SBUF
partitions; inputs are re-laid-out host-side so every device DMA is a single
contiguous block.  All matmuls run as float32r (full fp32 storage, reduced
PE mode, 1 cycle/row at free-dim >= 256 - 4x faster than plain fp32).
"""

import sys

sys.path.insert(0, "/opt/trn_rl_repo")

import numpy as np

from concourse import bacc, bass, mybir, tile
from concourse.bass_utils import run_bass_kernel_spmd

H = 512
DH = 4 * H            # 2048 (hidden feature dim)
B, S = 32, 2048
NCORES = 8
BC = B // NCORES      # 4 batch rows per core
KH = H // 128         # 4 contraction tiles over H
KD = DH // 128        # 16 contraction tiles over DH
NQ = H // 128         # 4 output quadrants of H
SBLK = 512            # sequence positions per block
NBLK = S // SBLK      # 4 blocks
F32 = mybir.dt.float32
F32R = mybir.dt.float32r
BF16 = mybir.dt.bfloat16
F8 = mybir.dt.float8e4
WE_SCALE = 64.0

_NC_CACHE = None


def _build():
    nc = bacc.Bacc(
        "TRN2", target_bir_lowering=False, debug=False, num_devices=NCORES
    )
    enc_d = nc.dram_tensor(
        "enc_t", [BC, NBLK, 128, KH, SBLK], F8, kind="ExternalInput"
    )
    hid_d = nc.dram_tensor("hid_t", [128, KD, BC], BF16, kind="ExternalInput")
    wh_d = nc.dram_tensor("w_h", [128, KD, H], BF16, kind="ExternalInput")
    we_d = nc.dram_tensor("w_e", [128, KH, H], F8, kind="ExternalInput")
    ba_d = nc.dram_tensor("b_attn", [128, NQ], F32, kind="ExternalInput")
    v_d = nc.dram_tensor("v", [128, NQ], BF16, kind="ExternalInput")
    id_d = nc.dram_tensor("ident", [BC, BC], F32, kind="ExternalInput")
    out_d = nc.dram_tensor("out", [BC, S], F32, kind="ExternalOutput")

    TANH = mybir.ActivationFunctionType.Tanh
    EXP = mybir.ActivationFunctionType.Exp
    COPY = mybir.ActivationFunctionType.Copy

    with tile.TileContext(nc) as tc:
        with (
            tc.tile_pool(name="const", bufs=1) as constp,
            tc.tile_pool(name="enc", bufs=6) as encp,
            tc.tile_pool(name="energy", bufs=9) as enp,
            tc.tile_pool(name="small", bufs=1) as smallp,
            tc.tile_pool(name="psum_e", bufs=7, space=bass.MemorySpace.PSUM) as pse,
            tc.tile_pool(name="psum_s", bufs=1, space=bass.MemorySpace.PSUM) as pss,
        ):
            wh_sb = constp.tile([128, KD, H], BF16)
            nc.scalar.dma_start(wh_sb[:, 0 : KD // 2, :], wh_d[:, 0 : KD // 2, :])
            we_sb = constp.tile([128, KH, H], F8)
            for k in range(KH):
                nc.scalar.dma_start(we_sb[:, k, :], we_d[:, k, :])
            ba_sb = constp.tile([128, NQ], F32)
            nc.scalar.dma_start(ba_sb[:], ba_d[:])
            v_sb = constp.tile([128, NQ], BF16)
            nc.scalar.dma_start(v_sb[:], v_d[:])
            id_sb = constp.tile([BC, BC], F32)
            nc.scalar.dma_start(id_sb[:], id_d[:])

            hptb = constp.tile([128, NQ, BC], F32)
            ex = smallp.tile([128, S], F32)
            out_sb = smallp.tile([128, S], F32)
            esum = smallp.tile([128, NBLK], F32)
            ssum = smallp.tile([128, 1], F32)
            rs = smallp.tile([128, 1], F32)

            ps_small = pss.tile([128, SBLK], F32)

            # HAM pre-warm: ~3.5 us of dummy matmuls on zeroed scratch while
            # the first DMAs are still in flight, so real matmuls start at
            # full clock (K=8/8)
            warm = constp.tile([128, 512], BF16)
            nc.vector.memset(warm[:], 0.0)
            for _ in range(8):
                nc.tensor.matmul(
                    ps_small[:, :], warm[:, 0:128], warm[:], start=True, stop=True
                )

            blocks = [(b, s) for b in range(BC) for s in range(NBLK)]
            NBLOCKS = len(blocks)
            ets = {}
            epss = {}

            def load_block(i):
                b, sblk = blocks[i]
                et = encp.tile([128, KH, SBLK], F8)
                nc.sync.dma_start(et[:], enc_d[b, sblk])
                ets[i] = et

            def emit_emm(i, qs=None):
                b, sblk = blocks[i]
                if qs is None or qs[0] == 0:
                    epss[i] = []
                et = ets[i]
                eps4 = epss[i]
                qlist = list(qs) if qs is not None else list(range(NQ))
                tiles = {}
                for q in qlist:
                    tiles[q] = pse.tile([128, SBLK], F32, name="eps", tag="eps")
                for qpair in [qlist[i : i + 2] for i in range(0, len(qlist), 2)]:
                    for j in range(KH // 2):
                        for q in qpair:
                            nc.tensor.matmul(
                                tiles[q][:],
                                we_sb[:, 2 * j : 2 * j + 2, q * 128 : (q + 1) * 128],
                                et[:, 2 * j : 2 * j + 2, :],
                                start=(j == 0),
                                stop=(j == KH // 2 - 1),
                                perf_mode=mybir.MatmulPerfMode.DoubleRow,
                            )
                for q in qlist:
                    eps4.append(tiles[q])
                if qs is None or qs[-1] == NQ - 1:
                    ets.pop(i)

            ens = {}

            def emit_tanh(i):
                b, sblk = blocks[i]
                en4 = []
                for q in range(NQ):
                    eps = epss[i][q]
                    en = enp.tile([128, SBLK], BF16)
                    nc.scalar.activation(
                        en[:],
                        eps[:],
                        TANH,
                        bias=hptb[:, q, b : b + 1],
                        scale=1.0 / WE_SCALE,
                    )
                    en4.append(en)
                ens[i] = en4
                del epss[i]

            def emit_v(i):
                att_ps = ps_small[(i % 3) * 32 : (i % 3) * 32 + 1, :]
                for q in range(NQ):
                    nc.tensor.matmul(
                        att_ps,
                        v_sb[:, q : q + 1],
                        ens[i][q][:],
                        start=(q == 0),
                        stop=(q == NQ - 1),
                    )
                del ens[i]

            def emit_exp(i):
                # exp of block i's logits (no max-sub: |logit| <= ||v||_1 ~ 18)
                # with a fused partial row-sum.  Deferred one block so it never
                # head-of-line-blocks the next block's tanh ops in the ACT FIFO.
                b, sblk = blocks[i]
                r0 = b * 32
                att_ps = ps_small[(i % 3) * 32 : (i % 3) * 32 + 1, :]
                if i >= NBLOCKS - 2:
                    # tail-critical: fused accumulator (290 ns) beats a
                    # separate 680 ns single-partition DVE reduce
                    nc.scalar.activation(
                        ex[r0 : r0 + 1, sblk * SBLK : (sblk + 1) * SBLK],
                        att_ps,
                        EXP,
                        accum_out=esum[r0 : r0 + 1, sblk : sblk + 1],
                    )
                else:
                    nc.scalar.activation(
                        ex[r0 : r0 + 1, sblk * SBLK : (sblk + 1) * SBLK],
                        att_ps,
                        EXP,
                    )
                    nc.vector.reduce_sum(
                        esum[r0 : r0 + 1, sblk : sblk + 1],
                        ex[r0 : r0 + 1, sblk * SBLK : (sblk + 1) * SBLK],
                        axis=mybir.AxisListType.X,
                    )
                if sblk == NBLK - 1:
                    # normalize row b as soon as its blocks are done
                    nc.vector.reduce_sum(
                        ssum[r0 : r0 + 1, :],
                        esum[r0 : r0 + 1, :],
                        axis=mybir.AxisListType.X,
                    )
                    nc.vector.reciprocal(rs[r0 : r0 + 1, :], ssum[r0 : r0 + 1, :])
                    if i == NBLOCKS - 1:
                        # last row: split across engines so the exposed tail
                        # is half as long
                        hs = S // 2
                        nc.vector.tensor_scalar_mul(
                            out_sb[r0 : r0 + 1, 0:hs],
                            ex[r0 : r0 + 1, 0:hs],
                            rs[r0 : r0 + 1, :],
                        )
                        nc.scalar.activation(
                            out_sb[r0 : r0 + 1, hs:S],
                            ex[r0 : r0 + 1, hs:S],
                            COPY,
                            scale=rs[r0 : r0 + 1, :],
                        )
                        nc.sync.dma_start(
                            out_d[b : b + 1, 0:hs], out_sb[r0 : r0 + 1, 0:hs]
                        )
                        nc.scalar.dma_start(
                            out_d[b : b + 1, hs:S], out_sb[r0 : r0 + 1, hs:S]
                        )
                    else:
                        nc.vector.tensor_scalar_mul(
                            out_sb[r0 : r0 + 1, :],
                            ex[r0 : r0 + 1, :],
                            rs[r0 : r0 + 1, :],
                        )
                        nc.sync.dma_start(
                            out_d[b : b + 1, :], out_sb[r0 : r0 + 1, :]
                        )

            # prologue: sync queue carries only enc tiles (fp8, 256 KB each);
            # h_part matmuls interleave with block 0's e-matmuls so the tanh
            # bias is ready as early as possible
            load_block(0)
            hid_sb = constp.tile([128, KD, BC], BF16)
            nc.sync.dma_start(hid_sb[:], hid_d[:])
            nc.sync.dma_start(wh_sb[:, KD // 2 :, :], wh_d[:, KD // 2 :, :])
            load_block(1)
            hp_ps = ps_small[0:BC, 0:H]

            def emit_hp(ks):
                for k in ks:
                    nc.tensor.matmul(
                        hp_ps,
                        hid_sb[:, k, :],
                        wh_sb[:, k, :],
                        start=(k == 0),
                        stop=(k == KD - 1),
                    )

            emit_hp(range(KD))
            hp_sb = smallp.tile([BC, H], F32)
            nc.vector.tensor_copy(hp_sb[:], hp_ps)

            # transpose to [128, q, b] via PE, fold in b_attn -> tanh bias
            for q in range(NQ):
                hpt_ps = ps_small[:, q * BC : (q + 1) * BC]
                nc.tensor.transpose(
                    hpt_ps, hp_sb[:, q * 128 : (q + 1) * 128], id_sb[:]
                )
                nc.vector.tensor_scalar_add(
                    hptb[:, q, :], hpt_ps, ba_sb[:, q : q + 1]
                )
            emit_emm(0)

            # steady state, one-block skew: e-matmuls of block i+1 sit ahead of
            # block i's tanh-dependent v-dots in the PE stream
            for i in range(NBLOCKS):
                if i + 2 < NBLOCKS:
                    load_block(i + 2)
                if i + 1 < NBLOCKS:
                    emit_emm(i + 1)
                emit_tanh(i)
                if i >= 1:
                    emit_v(i - 1)
                if i >= 2:
                    emit_exp(i - 2)
            emit_exp(NBLOCKS - 2)
            emit_v(NBLOCKS - 1)
            emit_exp(NBLOCKS - 1)

    nc.compile()
    return nc


def _get_nc():
    global _NC_CACHE
    if _NC_CACHE is None:
        _NC_CACHE = _build()
    return _NC_CACHE


def _prep_inputs(hidden, encoder_outputs, W_attn, b_attn, v):
    f = np.float32
    W_h = np.asarray(W_attn[:DH], dtype=f)
    W_e = np.asarray(W_attn[DH:], dtype=f)
    import ml_dtypes
    bf = ml_dtypes.bfloat16
    f8 = ml_dtypes.float8_e4m3
    wh_prep = np.ascontiguousarray(W_h.reshape(KD, 128, H).transpose(1, 0, 2)).astype(bf)
    we_prep = np.clip(
        np.ascontiguousarray(W_e.reshape(KH, 128, H).transpose(1, 0, 2)) * 64.0,
        -240.0, 240.0,
    ).astype(f8)
    ba_prep = np.ascontiguousarray(np.asarray(b_attn, dtype=f).reshape(NQ, 128).T)
    v_prep = np.ascontiguousarray(np.asarray(v, dtype=f).reshape(NQ, 128).T).astype(bf)
    ident = np.eye(BC, dtype=f)
    hidden = np.asarray(hidden, dtype=f)
    encoder_outputs = np.asarray(encoder_outputs, dtype=f)

    in_maps = []
    for c in range(NCORES):
        b0 = c * BC
        hc = hidden[b0 : b0 + BC]                       # [BC, DH]
        hid_prep = np.ascontiguousarray(
            hc.T.reshape(KD, 128, BC).transpose(1, 0, 2)
        ).astype(bf)
        ec = encoder_outputs[:, b0 : b0 + BC, :]        # [S, BC, H]
        # enc_prep[b, sblk, p, k, si] = ec[sblk*SBLK+si, b, k*128+p]
        enc_prep = np.clip(
            np.ascontiguousarray(
                ec.transpose(1, 0, 2)
                .reshape(BC, NBLK, SBLK, KH, 128)
                .transpose(0, 1, 4, 3, 2)
            ),
            -240.0, 240.0,
        ).astype(ml_dtypes.float8_e4m3)
        in_maps.append(
            {
                "enc_t": enc_prep,
                "hid_t": hid_prep,
                "w_h": wh_prep,
                "w_e": we_prep,
                "b_attn": ba_prep,
                "v": v_prep,
                "ident": ident,
            }
        )
    return in_maps


def _run(inputs, trace=False, **kw):
    nc = _get_nc()
    in_maps = _prep_inputs(
        inputs["hidden"],
        inputs["encoder_outputs"],
        inputs["W_attn"],
        inputs["b_attn"],
        inputs["v"],
    )
    res = run_bass_kernel_spmd(
        nc, in_maps, core_ids=list(range(NCORES)), trace=trace, **kw
    )
    out = np.concatenate([r["out"] for r in res.results], axis=0).astype(np.float32)
    return out, res


def kernel(**inputs):
    out, _ = _run(inputs, trace=False)
    return out


# revision 42
# speedup vs baseline: 1.1415x; 1.0191x over previous
"""Bahdanau-attention kernel for one TRN2 chip (8 NeuronCores, SPMD).

Math (per batch row b, sequence position s):
    att[b, s] = v . tanh(h_part[b] + enc[s, b, :] @ W_e)
    out[b, :] = softmax(att[b, :])        with h_part = hidden @ W_h + b_attn

Sharding: pure data-parallel over batch (B=32 -> 4 per core), no collectives.
Layout strategy: the big matmul contracts over H, which must live on SBUF
partitions; inputs are re-laid-out host-side so every device DMA is a single
contiguous block.  All matmuls run as float32r (full fp32 storage, reduced
PE mode, 1 cycle/row at free-dim >= 256 - 4x faster than plain fp32).
"""

import sys

sys.path.insert(0, "/opt/trn_rl_repo")

import numpy as np

from concourse import bacc, bass, mybir, tile
from concourse.bass_utils import run_bass_kernel_spmd

H = 512
DH = 4 * H            # 2048 (hidden feature dim)
B, S = 32, 2048
NCORES = 8
BC = B // NCORES      # 4 batch rows per core
KH = H // 128         # 4 contraction tiles over H
KD = DH // 128        # 16 contraction tiles over DH
NQ = H // 128         # 4 output quadrants of H
SBLK = 1024           # sequence positions per block
NBLK = S // SBLK      # 2 blocks per batch row
HB = 512              # half-block: psum-bank / matmul-N granularity
NCH = S // HB         # 4 per-row chunks for the softmax
F32 = mybir.dt.float32
F32R = mybir.dt.float32r
BF16 = mybir.dt.bfloat16
F8 = mybir.dt.float8e4
WE_SCALE = 64.0

_NC_CACHE = None


def _build():
    nc = bacc.Bacc(
        "TRN2", target_bir_lowering=False, debug=False, num_devices=NCORES
    )
    enc_d = nc.dram_tensor(
        "enc_t", [BC, NBLK, 128, KH, SBLK], F8, kind="ExternalInput"
    )
    hid_d = nc.dram_tensor("hid_t", [128, KD, BC], BF16, kind="ExternalInput")
    wh_d = nc.dram_tensor("w_h", [128, KD, H], BF16, kind="ExternalInput")
    we_d = nc.dram_tensor("w_e", [128, KH, H], F8, kind="ExternalInput")
    ba_d = nc.dram_tensor("b_attn", [128, NQ], F32, kind="ExternalInput")
    v_d = nc.dram_tensor("v", [128, NQ], BF16, kind="ExternalInput")
    id_d = nc.dram_tensor("ident", [BC, BC], F32, kind="ExternalInput")
    out_d = nc.dram_tensor("out", [BC, S], F32, kind="ExternalOutput")

    TANH = mybir.ActivationFunctionType.Tanh
    EXP = mybir.ActivationFunctionType.Exp
    COPY = mybir.ActivationFunctionType.Copy

    with tile.TileContext(nc) as tc:
        with (
            tc.tile_pool(name="const", bufs=1) as constp,
            tc.tile_pool(name="enc", bufs=6) as encp,
            tc.tile_pool(name="energy", bufs=8) as enp,
            tc.tile_pool(name="small", bufs=1) as smallp,
            tc.tile_pool(name="psum_e", bufs=3, space=bass.MemorySpace.PSUM) as pse,
            tc.tile_pool(name="psum_s", bufs=1, space=bass.MemorySpace.PSUM) as pss,
        ):
            wh_sb = constp.tile([128, KD, H], BF16)
            nc.scalar.dma_start(wh_sb[:, 0 : KD // 2, :], wh_d[:, 0 : KD // 2, :])
            we_sb = constp.tile([128, KH, H], F8)
            for k in range(KH):
                nc.scalar.dma_start(we_sb[:, k, :], we_d[:, k, :])
            ba_sb = constp.tile([128, NQ], F32)
            nc.scalar.dma_start(ba_sb[:], ba_d[:])
            v_sb = constp.tile([128, NQ], BF16)
            nc.scalar.dma_start(v_sb[:], v_d[:])
            id_sb = constp.tile([BC, BC], F32)
            nc.scalar.dma_start(id_sb[:], id_d[:])

            hptb = constp.tile([128, NQ, BC], F32)
            ex = smallp.tile([128, S], F32)
            out_sb = smallp.tile([128, S], F32)
            esum = smallp.tile([128, NCH], F32)
            ssum = smallp.tile([128, 1], F32)
            rs = smallp.tile([128, 1], F32)

            ps_small = pss.tile([128, HB], F32)

            # HAM pre-warm: ~3.5 us of dummy matmuls on zeroed scratch while
            # the first DMAs are still in flight, so real matmuls start at
            # full clock (K=8/8)
            warm = constp.tile([128, 512], BF16)
            nc.vector.memset(warm[:], 0.0)
            for _ in range(8):
                nc.tensor.matmul(
                    ps_small[:, :], warm[:, 0:128], warm[:], start=True, stop=True
                )

            blocks = [(b, s) for b in range(BC) for s in range(NBLK)]
            NBLOCKS = len(blocks)
            ets = {}
            epss = {}

            def load_block(i):
                b, sblk = blocks[i]
                et = encp.tile([128, KH, SBLK], F8)
                nc.sync.dma_start(et[:], enc_d[b, sblk])
                ets[i] = et

            def emit_emm(i, qs=None):
                b, sblk = blocks[i]
                if qs is None or qs[0] == 0:
                    epss[i] = []
                et = ets[i]
                eps4 = epss[i]
                qlist = list(qs) if qs is not None else list(range(NQ))
                tiles = {}
                for q in qlist:
                    tiles[q] = pse.tile([128, SBLK], F32, name="eps", tag="eps")
                for qpair in [qlist[i : i + 2] for i in range(0, len(qlist), 2)]:
                    for half in range(SBLK // HB):
                        hsl = slice(half * HB, (half + 1) * HB)
                        for j in range(KH // 2):
                            for q in qpair:
                                nc.tensor.matmul(
                                    tiles[q][:, hsl],
                                    we_sb[
                                        :, 2 * j : 2 * j + 2, q * 128 : (q + 1) * 128
                                    ],
                                    et[:, 2 * j : 2 * j + 2, hsl],
                                    start=(j == 0),
                                    stop=(j == KH // 2 - 1),
                                    perf_mode=mybir.MatmulPerfMode.DoubleRow,
                                )
                for q in qlist:
                    eps4.append(tiles[q])
                if qs is None or qs[-1] == NQ - 1:
                    ets.pop(i)

            ens = {}

            def emit_tanh(i):
                b, sblk = blocks[i]
                en4 = []
                for q in range(NQ):
                    eps = epss[i][q]
                    en = enp.tile([128, SBLK], BF16)
                    nc.scalar.activation(
                        en[:],
                        eps[:],
                        TANH,
                        bias=hptb[:, q, b : b + 1],
                        scale=1.0 / WE_SCALE,
                    )
                    en4.append(en)
                ens[i] = en4
                del epss[i]

            def emit_v(i):
                for half in range(SBLK // HB):
                    c = i * (SBLK // HB) + half
                    att_ps = ps_small[(c % 3) * 32 : (c % 3) * 32 + 1, 0:HB]
                    for q in range(NQ):
                        nc.tensor.matmul(
                            att_ps,
                            v_sb[:, q : q + 1],
                            ens[i][q][:, half * HB : (half + 1) * HB],
                            start=(q == 0),
                            stop=(q == NQ - 1),
                        )
                del ens[i]

            def emit_exp(i):
                # exp of block i's logits (no max-sub: |logit| <= ||v||_1 ~ 18).
                # Deferred so it never head-of-line-blocks tanh in the ACT FIFO.
                b, sblk = blocks[i]
                r0 = b * 32
                for half in range(SBLK // HB):
                    emit_exp_chunk(i, b, sblk * (SBLK // HB) + half,
                                   i * (SBLK // HB) + half)
                if sblk == NBLK - 1:
                    emit_norm(i, b, r0)

            def emit_exp_chunk(i, b, c, cg):
                r0 = b * 32
                att_ps = ps_small[(cg % 3) * 32 : (cg % 3) * 32 + 1, 0:HB]
                if i >= NBLOCKS - 2:
                    # tail-critical: fused accumulator (290 ns) beats a
                    # separate 680 ns single-partition DVE reduce
                    nc.scalar.activation(
                        ex[r0 : r0 + 1, c * HB : (c + 1) * HB],
                        att_ps,
                        EXP,
                        accum_out=esum[r0 : r0 + 1, c : c + 1],
                    )
                else:
                    nc.scalar.activation(
                        ex[r0 : r0 + 1, c * HB : (c + 1) * HB],
                        att_ps,
                        EXP,
                    )
                    nc.vector.reduce_sum(
                        esum[r0 : r0 + 1, c : c + 1],
                        ex[r0 : r0 + 1, c * HB : (c + 1) * HB],
                        axis=mybir.AxisListType.X,
                    )

            def emit_norm(i, b, r0):
                if True:
                    # normalize row b as soon as its blocks are done
                    nc.vector.reduce_sum(
                        ssum[r0 : r0 + 1, :],
                        esum[r0 : r0 + 1, :],
                        axis=mybir.AxisListType.X,
                    )
                    nc.vector.reciprocal(rs[r0 : r0 + 1, :], ssum[r0 : r0 + 1, :])
                    if i == NBLOCKS - 1:
                        # last row: split across engines so the exposed tail
                        # is half as long
                        hs = S // 2
                        nc.vector.tensor_scalar_mul(
                            out_sb[r0 : r0 + 1, 0:hs],
                            ex[r0 : r0 + 1, 0:hs],
                            rs[r0 : r0 + 1, :],
                        )
                        nc.scalar.activation(
                            out_sb[r0 : r0 + 1, hs:S],
                            ex[r0 : r0 + 1, hs:S],
                            COPY,
                            scale=rs[r0 : r0 + 1, :],
                        )
                        nc.sync.dma_start(
                            out_d[b : b + 1, 0:hs], out_sb[r0 : r0 + 1, 0:hs]
                        )
                        nc.scalar.dma_start(
                            out_d[b : b + 1, hs:S], out_sb[r0 : r0 + 1, hs:S]
                        )
                    else:
                        nc.vector.tensor_scalar_mul(
                            out_sb[r0 : r0 + 1, :],
                            ex[r0 : r0 + 1, :],
                            rs[r0 : r0 + 1, :],
                        )
                        nc.sync.dma_start(
                            out_d[b : b + 1, :], out_sb[r0 : r0 + 1, :]
                        )

            # prologue: sync queue carries only enc tiles (fp8, 256 KB each);
            # h_part matmuls interleave with block 0's e-matmuls so the tanh
            # bias is ready as early as possible
            load_block(0)
            hid_sb = constp.tile([128, KD, BC], BF16)
            nc.sync.dma_start(hid_sb[:], hid_d[:])
            nc.sync.dma_start(wh_sb[:, KD // 2 :, :], wh_d[:, KD // 2 :, :])
            load_block(1)
            hp_ps = ps_small[0:BC, 0:H]

            def emit_hp(ks):
                for k in ks:
                    nc.tensor.matmul(
                        hp_ps,
                        hid_sb[:, k, :],
                        wh_sb[:, k, :],
                        start=(k == 0),
                        stop=(k == KD - 1),
                    )

            emit_hp(range(KD))
            hp_sb = smallp.tile([BC, H], F32)
            nc.vector.tensor_copy(hp_sb[:], hp_ps)

            # transpose to [128, q, b] via PE, fold in b_attn -> tanh bias
            for q in range(NQ):
                hpt_ps = ps_small[:, q * BC : (q + 1) * BC]
                nc.tensor.transpose(
                    hpt_ps, hp_sb[:, q * 128 : (q + 1) * 128], id_sb[:]
                )
                nc.vector.tensor_scalar_add(
                    hptb[:, q, :], hpt_ps, ba_sb[:, q : q + 1]
                )
            emit_emm(0)

            # steady state, one-block skew: e-matmuls of block i+1 sit ahead of
            # block i's tanh-dependent v-dots in the PE stream
            for i in range(NBLOCKS):
                if i + 2 < NBLOCKS:
                    load_block(i + 2)
                if i + 1 < NBLOCKS:
                    emit_emm(i + 1)
                emit_tanh(i)
                if i >= 1:
                    emit_v(i - 1)
                    emit_exp(i - 1)
            emit_v(NBLOCKS - 1)
            emit_exp(NBLOCKS - 1)

    nc.compile()
    return nc


def _get_nc():
    global _NC_CACHE
    if _NC_CACHE is None:
        _NC_CACHE = _build()
    return _NC_CACHE


def _prep_inputs(hidden, encoder_outputs, W_attn, b_attn, v):
    f = np.float32
    W_h = np.asarray(W_attn[:DH], dtype=f)
    W_e = np.asarray(W_attn[DH:], dtype=f)
    import ml_dtypes
    bf = ml_dtypes.bfloat16
    f8 = ml_dtypes.float8_e4m3
    wh_prep = np.ascontiguousarray(W_h.reshape(KD, 128, H).transpose(1, 0, 2)).astype(bf)
    we_prep = np.clip(
        np.ascontiguousarray(W_e.reshape(KH, 128, H).transpose(1, 0, 2)) * 64.0,
        -240.0, 240.0,
    ).astype(f8)
    ba_prep = np.ascontiguousarray(np.asarray(b_attn, dtype=f).reshape(NQ, 128).T)
    v_prep = np.ascontiguousarray(np.asarray(v, dtype=f).reshape(NQ, 128).T).astype(bf)
    ident = np.eye(BC, dtype=f)
    hidden = np.asarray(hidden, dtype=f)
    encoder_outputs = np.asarray(encoder_outputs, dtype=f)

    in_maps = []
    for c in range(NCORES):
        b0 = c * BC
        hc = hidden[b0 : b0 + BC]                       # [BC, DH]
        hid_prep = np.ascontiguousarray(
            hc.T.reshape(KD, 128, BC).transpose(1, 0, 2)
        ).astype(bf)
        ec = encoder_outputs[:, b0 : b0 + BC, :]        # [S, BC, H]
        # enc_prep[b, sblk, p, k, si] = ec[sblk*SBLK+si, b, k*128+p]
        enc_prep = np.clip(
            np.ascontiguousarray(
                ec.transpose(1, 0, 2)
                .reshape(BC, NBLK, SBLK, KH, 128)
                .transpose(0, 1, 4, 3, 2)
            ),
            -240.0, 240.0,
        ).astype(ml_dtypes.float8_e4m3)
        in_maps.append(
            {
                "enc_t": enc_prep,
                "hid_t": hid_prep,
                "w_h": wh_prep,
                "w_e": we_prep,
                "b_attn": ba_prep,
                "v": v_prep,
                "ident": ident,
            }
        )
    return in_maps


def _run(inputs, trace=False, **kw):
    nc = _get_nc()
    in_maps = _prep_inputs(
        inputs["hidden"],
        inputs["encoder_outputs"],
        inputs["W_attn"],
        inputs["b_attn"],
        inputs["v"],
    )
    res = run_bass_kernel_spmd(
        nc, in_maps, core_ids=list(range(NCORES)), trace=trace, **kw
    )
    out = np.concatenate([r["out"] for r in res.results], axis=0).astype(np.float32)
    return out, res


def kernel(**inputs):
    out, _ = _run(inputs, trace=False)
    return out
